# revision 8
# baseline (speedup 1.0000x reference)
"""Block-sparse position-wise FFN on Trainium2 (Bass/Tile), 8-core data-parallel.

Strategy (v3 — dense bf16 streaming + permutation-based block skipping):
  - Shard tokens (B*S = 36928) across 8 cores: 4616 tokens/core. Pointwise
    FFN + weights fit in SBUF => data-parallel, no collectives.
  - All device data is bf16 (PSUM accumulation fp32). bf16 streams at
    1 cycle/row at ANY free size and enables fast-weight-load, so the
    per-matmul LDWEIGHTS (~53ns) hides fully under N=512 matmuls (~213ns).
  - Host pre-transposes x; the device consumes xT [768, T] directly (no PE
    transposes). Both layers keep weights stationary:
      fc1: hT[m]   = gelu(w1t[k][:, m].T @ xT[k]  + b1), accumulate over k
      fc2: outT[o] =      w2t[k][:, o].T @ hT[k] + b2,  accumulate over k
    Output is written transposed [768, T]; host untransposes (free).
  - Sparsity: random 10%-dense 8x8 blocks aggregate to ~80% density at any
    128-wide PE tile, so generic skipping is impossible. BUT a host-chosen
    global permutation of ff/dim/out BLOCKS can pack mask-dead rows into
    whole 128x128 stationary tiles: a greedy co-clustering packs f-blocks
    that share a dead contraction k-tile into the same m-tile (and o-blocks
    likewise), making ~20+ of the 288 stationary tiles exactly zero =>
    those matmuls are simply not emitted (~7% less PE work).
"""

import sys
import types

import numpy as np
import ml_dtypes

# concourse's axon trace path imports antenv.axon_hooks, which this image
# lacks; install a no-op shim so an env-requested trace degrades gracefully
# instead of raising ImportError.
try:
    import antenv.axon_hooks  # noqa: F401
except ImportError:
    import antenv

    _hooks = types.ModuleType("antenv.axon_hooks")
    _hooks._hook = None
    _hooks.set_axon_ntff_profile_hook = (
        lambda h: setattr(_hooks, "_hook", h))
    _hooks.get_axon_ntff_profile_hook = lambda: _hooks._hook
    sys.modules["antenv.axon_hooks"] = _hooks
    antenv.axon_hooks = _hooks

import concourse.bass as bass
import concourse.bacc as bacc
import concourse.mybir as mybir
from concourse import tile
from concourse.bass_utils import run_bass_kernel_spmd

B, S, DIM, FF, BLK = 64, 577, 768, 3072, 8
NCORES = 8
TOK = B * S                # 36928
T = TOK // NCORES          # 4616 tokens per core
P = 128
KD = DIM // P              # 6 contraction tiles for fc1 / output tiles fc2
KF = FF // P               # 24 ff tiles
CW = 512                   # chunk width (one PSUM bank of fp32)
F32 = mybir.dt.float32
BF16 = mybir.dt.bfloat16
GELU = mybir.ActivationFunctionType.Gelu

# 8x512 + 344 + 176 = 4616; all chunks wide enough to amortize dispatch,
# small final chunk shortens the post-compute drain tail
CHUNKS = [512] * 8 + [344, 176]
assert sum(CHUNKS) == T

NF, ND, NO = FF // BLK, DIM // BLK, DIM // BLK   # 384, 96, 96 blocks
FT, DT, OT = KF, KD, KD                          # 24, 6, 6 tiles
BPT = P // BLK                                   # 16 blocks per tile


# ---------------------------------------------------------------------------
# Host-side permutation search: pack mask-dead blocks into whole zero tiles.
# ---------------------------------------------------------------------------

def _greedy_fgroups(dead1, rng=None):
    """dead1 [NF, DT] bool -> f-block -> m-tile, packing whole dead tiles."""
    fg = -np.ones(NF, np.int32)
    tilei = 0
    remaining = np.ones(NF, bool)
    # tiles dead for a PAIR of k's first (worth 2 skips each)
    pairs = [(ka, kb) for ka in range(DT) for kb in range(ka + 1, DT)]
    if rng is not None:
        rng.shuffle(pairs)
    for ka, kb in pairs:
        while tilei < FT:
            cand = np.where(remaining & dead1[:, ka] & dead1[:, kb])[0]
            if len(cand) < BPT:
                break
            pick = cand[:BPT]
            fg[pick] = tilei
            remaining[pick] = False
            tilei += 1
    # single-k tiles, k by descending availability
    while tilei < FT:
        counts = sorted(((dead1[remaining, k].sum(), k) for k in range(DT)),
                        reverse=True)
        n, k = counts[0]
        if n < BPT:
            break
        cand = np.where(remaining & dead1[:, k])[0]
        other = dead1[cand].sum(1)   # prefer blocks with fewest other dead-k
        if rng is not None:
            order = np.argsort(other + rng.random(len(cand)) * 0.5)
        else:
            order = np.argsort(other, kind="stable")
        pick = cand[order][:BPT]
        fg[pick] = tilei
        remaining[pick] = False
        tilei += 1
    left = np.where(remaining)[0]
    pos = 0
    for t in range(FT):
        space = BPT - int((fg == t).sum())
        if space > 0:
            fg[left[pos:pos + space]] = t
            pos += space
    return fg


def _greedy_ogroups(dead2):
    """dead2 [NO, FT] bool -> o-block -> o-tile (6 tiles of 16)."""
    og = -np.ones(NO, np.int32)
    remaining = np.ones(NO, bool)
    tilei = 0
    counts = sorted(((dead2[:, ft].sum(), ft) for ft in range(FT)),
                    reverse=True)
    for n, ft in counts:
        if tilei >= OT:
            break
        cand = np.where(remaining & dead2[:, ft])[0]
        if len(cand) < BPT:
            continue
        other = dead2[cand].sum(1)
        pick = cand[np.argsort(other, kind="stable")][:BPT]
        og[pick] = tilei
        remaining[pick] = False
        tilei += 1
    left = np.where(remaining)[0]
    pos = 0
    for t in range(OT):
        space = BPT - int((og == t).sum())
        if space > 0:
            og[left[pos:pos + space]] = t
            pos += space
    return og


def _count_alive(m1, m2, fg, dg, og):
    nz1 = np.stack([m1[:, dg == k].sum(1) for k in range(DT)], 1)
    nz2 = np.stack([m2[:, fg == t].sum(1) for t in range(FT)], 1)
    alive1 = np.stack([(nz1[fg == t] > 0).sum(0) for t in range(FT)])
    alive2 = np.stack([(nz2[og == t] > 0).sum(0) for t in range(OT)])
    return alive1, alive2


def _repair(m1, m2, fg, dg, og, max_rounds=40):
    """Complete nearly-dead fc1 tiles via constrained f-block swaps that
    preserve every already-empty tile (fc1 and fc2)."""
    dead1 = np.stack([~m1[:, dg == k].any(1) for k in range(DT)], 1)
    for _ in range(max_rounds):
        alive1, alive2 = _count_alive(m1, m2, fg, dg, og)
        skipset1 = [set(np.where(alive1[t] == 0)[0]) for t in range(FT)]
        skipset2 = [set(np.where(alive2[t] == 0)[0]) for t in range(OT)]
        orows_req = [set() for _ in range(FT)]
        for ot in range(OT):
            for ft in skipset2[ot]:
                orows_req[ft].update(np.where(og == ot)[0])
        improved = False
        order = sorted(
            (int(alive1[mt, kt]), mt, kt)
            for mt in range(FT) for kt in range(DT)
            if 1 <= alive1[mt, kt] <= 3)
        for _na, mt, kt in order:
            alive_blocks = [f for f in np.where(fg == mt)[0]
                            if not dead1[f, kt]]
            req_mt = skipset1[mt] | {kt}
            oreq_mt = orows_req[mt]
            swaps, used_out, ok = [], set(), True
            for a in alive_blocks:
                found = None
                for b in np.where(dead1[:, kt])[0]:
                    if fg[b] == mt or b in used_out:
                        continue
                    tb = fg[b]
                    if not all(dead1[b, k2] for k2 in req_mt):
                        continue
                    if any(m2[o, b] for o in oreq_mt):
                        continue
                    if not all(dead1[a, k2] for k2 in skipset1[tb]):
                        continue
                    if any(m2[o, a] for o in orows_req[tb]):
                        continue
                    found = b
                    break
                if found is None:
                    ok = False
                    break
                used_out.add(found)
                swaps.append((a, found))
            if ok and swaps:
                for a, b in swaps:
                    fg[a], fg[b] = fg[b], fg[a]
                improved = True
                break
        if not improved:
            break
    return fg


def plan_permutation(mask1, mask2, restarts=30):
    m1 = np.asarray(mask1, bool)   # [NF, ND]
    m2 = np.asarray(mask2, bool)   # [NO, NF]
    dg = np.arange(ND) // BPT      # natural d-grouping
    dead1 = np.stack([~m1[:, dg == k].any(1) for k in range(DT)], 1)
    best = None
    for trial in range(restarts):
        rng = None if trial == 0 else np.random.default_rng(trial)
        fg = _greedy_fgroups(dead1, rng=rng)
        fg = _repair(m1, m2, fg, dg, np.arange(NO) // BPT)
        dead2 = np.stack([~m2[:, fg == t].any(1) for t in range(FT)], 1)
        og = _greedy_ogroups(dead2)
        fg = _repair(m1, m2, fg, dg, og)
        a1, a2 = _count_alive(m1, m2, fg, dg, og)
        score = int((a1 == 0).sum() + (a2 == 0).sum())
        if best is None or score > best[0]:
            best = (score, fg.copy(), og.copy())
    _, fg, og = best

    def perm(g, ntiles):
        return np.concatenate([np.where(g == t)[0] for t in range(ntiles)])

    fperm, dperm, operm = perm(fg, FT), perm(dg, DT), perm(og, OT)
    p1 = m1[np.ix_(fperm, dperm)]
    p2 = m2[np.ix_(operm, fperm)]
    skip1 = frozenset(
        (mt, kt) for mt in range(FT) for kt in range(DT)
        if not p1[mt*BPT:(mt+1)*BPT, kt*BPT:(kt+1)*BPT].any())
    skip2 = frozenset(
        (ot, ft) for ot in range(OT) for ft in range(FT)
        if not p2[ot*BPT:(ot+1)*BPT, ft*BPT:(ft+1)*BPT].any())

    def expand(p):
        return (p[:, None] * BLK + np.arange(BLK)[None, :]).ravel()

    return expand(fperm), expand(dperm), expand(operm), skip1, skip2


# ---------------------------------------------------------------------------
# Device program
# ---------------------------------------------------------------------------

def _body(tc, xt_d, w1_d, b1_d, w2_d, b2_d, o_d, skip1, skip2):
    nc = tc.nc
    with (
        tc.tile_pool(name="const", bufs=1) as constp,
        tc.tile_pool(name="wpool", bufs=1) as wp,
        tc.tile_pool(name="xt", bufs=2) as xtp,
        tc.tile_pool(name="h", bufs=2) as hp,
        tc.tile_pool(name="onat", bufs=2) as onatp,
        tc.tile_pool(name="ps1", bufs=3, space=bass.MemorySpace.PSUM) as ps1p,
        tc.tile_pool(name="ps2", bufs=3, space=bass.MemorySpace.PSUM) as ps2p,
    ):
        b1_s = constp.tile([P, KF], F32)
        nc.sync.dma_start(out=b1_s[:], in_=b1_d)
        b2_s = constp.tile([P, KD], F32)
        nc.sync.dma_start(out=b2_s[:], in_=b2_d)

        # w1 arrival order: the first two m-tiles' 128-col slices first (so
        # fc1 starts within ~2us), then the remainder in column-major
        # quarters. w2 streams concurrently on the scalar engine's DMA queue.
        w1_s = [wp.tile([P, FF], BF16, tag=f"w1_{k}", name=f"w1_{k}")
                for k in range(KD)]
        for m in range(2):
            for k in range(KD):
                nc.sync.dma_start(
                    out=w1_s[k][:, m * P:(m + 1) * P],
                    in_=w1_d[k * P:(k + 1) * P, m * P:(m + 1) * P],
                )
        W1Q = (FF - 2 * P) // 4
        for q in range(4):
            lo = 2 * P + q * W1Q
            hi = 2 * P + (q + 1) * W1Q
            for k in range(KD):
                nc.sync.dma_start(
                    out=w1_s[k][:, lo:hi],
                    in_=w1_d[k * P:(k + 1) * P, lo:hi],
                )
        w2_s = []
        for k in range(KF):
            w = wp.tile([P, DIM], BF16, tag=f"w2_{k}", name=f"w2_{k}")
            nc.scalar.dma_start(out=w[:], in_=w2_d[k * P:(k + 1) * P, :])
            w2_s.append(w)

        def load_x(c0, cw):
            xts = [xtp.tile([P, CW], BF16, tag=f"xt{k}", name=f"xt{k}")
                   for k in range(KD)]
            for k in range(KD):
                nc.gpsimd.dma_start(
                    out=xts[k][:, 0:cw],
                    in_=xt_d[k * P:(k + 1) * P, c0:c0 + cw],
                )
            return xts

        fc1_ks = [[k for k in range(KD) if (m, k) not in skip1] or [0]
                  for m in range(KF)]
        fc2_ks = [[k for k in range(KF) if (o, k) not in skip2] or [0]
                  for o in range(KD)]

        starts = [sum(CHUNKS[:i]) for i in range(len(CHUNKS))]
        xts = load_x(starts[0], CHUNKS[0])
        for ci, (c0, cw) in enumerate(zip(starts, CHUNKS)):
            # prefetch next chunk's xT while this chunk computes
            next_xts = (load_x(starts[ci + 1], CHUNKS[ci + 1])
                        if ci + 1 < len(CHUNKS) else None)

            # --- fc1: hT[m] = gelu(W1m slice.T @ xT + b1[m]) ---
            hts = []
            for m in range(KF):
                ps = ps1p.tile([P, CW], F32, tag="ps1", name="ps1")
                ks = fc1_ks[m]
                for k in ks:
                    nc.tensor.matmul(
                        ps[:, 0:cw],
                        w1_s[k][:, m * P:(m + 1) * P],
                        xts[k][:, 0:cw],
                        start=(k == ks[0]), stop=(k == ks[-1]),
                    )
                ht = hp.tile([P, CW], BF16, tag=f"h{m}", name=f"h{m}")
                nc.scalar.activation(
                    ht[:, 0:cw], ps[:, 0:cw], GELU, bias=b1_s[:, m:m + 1]
                )
                hts.append(ht)

            # --- fc2: outT[o] = W2m slice.T @ hT + b2[o] ---
            for o in range(KD):
                ps = ps2p.tile([P, CW], F32, tag="ps2", name="ps2")
                ks = fc2_ks[o]
                for k in ks:
                    nc.tensor.matmul(
                        ps[:, 0:cw],
                        w2_s[k][:, o * P:(o + 1) * P],
                        hts[k][:, 0:cw],
                        start=(k == ks[0]), stop=(k == ks[-1]),
                    )
                ot = onatp.tile([P, CW], BF16, tag=f"o{o}", name=f"o{o}")
                nc.vector.tensor_scalar_add(
                    ot[:, 0:cw], ps[:, 0:cw], b2_s[:, o:o + 1]
                )
                nc.sync.dma_start(
                    out=o_d[o * P:(o + 1) * P, c0:c0 + cw], in_=ot[:, 0:cw]
                )
            xts = next_xts


def build_program(skip1, skip2, t_tokens=T):
    nc = bacc.Bacc("TRN2", target_bir_lowering=False, debug=False,
                   num_devices=NCORES)
    xt_d = nc.dram_tensor("xt", [DIM, t_tokens], BF16,
                          kind="ExternalInput").ap()
    w1_d = nc.dram_tensor("w1t", [DIM, FF], BF16, kind="ExternalInput").ap()
    b1_d = nc.dram_tensor("b1", [P, KF], F32, kind="ExternalInput").ap()
    w2_d = nc.dram_tensor("w2t", [FF, DIM], BF16, kind="ExternalInput").ap()
    b2_d = nc.dram_tensor("b2", [P, KD], F32, kind="ExternalInput").ap()
    o_d = nc.dram_tensor("outt", [DIM, t_tokens], BF16,
                         kind="ExternalOutput").ap()
    with tile.TileContext(nc) as tc:
        _body(tc, xt_d, w1_d, b1_d, w2_d, b2_d, o_d, skip1, skip2)
    nc.compile()
    return nc


def host_prep(x, W1, b1, W2, b2, mask1, mask2, fpe, dpe, ope):
    m1 = np.repeat(np.repeat(np.asarray(mask1, dtype=bool), BLK, 0), BLK, 1)
    m2 = np.repeat(np.repeat(np.asarray(mask2, dtype=bool), BLK, 0), BLK, 1)
    xt = np.ascontiguousarray(
        np.asarray(x, np.float32).reshape(TOK, DIM).T[dpe]
    ).astype(ml_dtypes.bfloat16)                                  # [DIM, TOK]
    wm1t = (np.asarray(W1, np.float32) * m1.astype(np.float32)).T
    w1t = np.ascontiguousarray(
        wm1t[np.ix_(dpe, fpe)]).astype(ml_dtypes.bfloat16)        # [DIM, FF]
    wm2t = (np.asarray(W2, np.float32) * m2.astype(np.float32)).T
    w2t = np.ascontiguousarray(
        wm2t[np.ix_(fpe, ope)]).astype(ml_dtypes.bfloat16)        # [FF, DIM]
    b1h = np.ascontiguousarray(
        np.asarray(b1, np.float32)[fpe].reshape(KF, P).T)         # [P, KF]
    b2h = np.ascontiguousarray(
        np.asarray(b2, np.float32)[ope].reshape(KD, P).T)         # [P, KD]
    return xt, w1t, b1h, w2t, b2h


_PROGRAM = None
_PROGRAM_KEY = None


def _get_program(skip1, skip2, key):
    global _PROGRAM, _PROGRAM_KEY
    if _PROGRAM is None or _PROGRAM_KEY != key:
        _PROGRAM = build_program(skip1, skip2)
        _PROGRAM_KEY = key
    return _PROGRAM


def kernel(x, W1, b1, W2, b2, mask1, mask2, **run_kwargs):
    fpe, dpe, ope, skip1, skip2 = plan_permutation(mask1, mask2)
    xt, w1t, b1h, w2t, b2h = host_prep(
        x, W1, b1, W2, b2, mask1, mask2, fpe, dpe, ope)
    key = (np.asarray(mask1).tobytes(), np.asarray(mask2).tobytes())
    nc = _get_program(skip1, skip2, key)
    in_maps = [
        {"xt": np.ascontiguousarray(xt[:, c * T:(c + 1) * T]),
         "w1t": w1t, "b1": b1h, "w2t": w2t, "b2": b2h}
        for c in range(NCORES)
    ]
    res = run_bass_kernel_spmd(nc, in_maps, list(range(NCORES)), **run_kwargs)
    outt = np.concatenate(
        [np.asarray(res.results[c]["outt"]) for c in range(NCORES)], axis=1
    )                                                             # [DIM, TOK]
    out = np.empty((TOK, DIM), np.float32)
    out[:, ope] = outt.T.astype(np.float32)
    out = out.reshape(B, S, DIM)
    if run_kwargs:
        kernel.last_results = res
    return out


# revision 9
# speedup vs baseline: 1.1844x; 1.1844x over previous
"""Block-sparse position-wise FFN on Trainium2 (Bass/Tile), 8-core data-parallel.

Strategy (v3 — dense bf16 streaming + permutation-based block skipping):
  - Shard tokens (B*S = 36928) across 8 cores: 4616 tokens/core. Pointwise
    FFN + weights fit in SBUF => data-parallel, no collectives.
  - All device data is bf16 (PSUM accumulation fp32). bf16 streams at
    1 cycle/row at ANY free size and enables fast-weight-load, so the
    per-matmul LDWEIGHTS (~53ns) hides fully under N=512 matmuls (~213ns).
  - Host pre-transposes x; the device consumes xT [768, T] directly (no PE
    transposes). Both layers keep weights stationary:
      fc1: hT[m]   = gelu(w1t[k][:, m].T @ xT[k]  + b1), accumulate over k
      fc2: outT[o] =      w2t[k][:, o].T @ hT[k] + b2,  accumulate over k
    Output is written transposed [768, T]; host untransposes (free).
  - Sparsity: random 10%-dense 8x8 blocks aggregate to ~80% density at any
    128-wide PE tile, so generic skipping is impossible. BUT a host-chosen
    global permutation of ff/dim/out BLOCKS can pack mask-dead rows into
    whole 128x128 stationary tiles: a greedy co-clustering packs f-blocks
    that share a dead contraction k-tile into the same m-tile (and o-blocks
    likewise), making ~20+ of the 288 stationary tiles exactly zero =>
    those matmuls are simply not emitted (~7% less PE work).
"""

import sys
import types

import numpy as np
import ml_dtypes

# concourse's axon trace path imports antenv.axon_hooks, which this image
# lacks; install a no-op shim so an env-requested trace degrades gracefully
# instead of raising ImportError.
try:
    import antenv.axon_hooks  # noqa: F401
except ImportError:
    import antenv

    _hooks = types.ModuleType("antenv.axon_hooks")
    _hooks._hook = None
    _hooks.set_axon_ntff_profile_hook = (
        lambda h: setattr(_hooks, "_hook", h))
    _hooks.get_axon_ntff_profile_hook = lambda: _hooks._hook
    sys.modules["antenv.axon_hooks"] = _hooks
    antenv.axon_hooks = _hooks

import concourse.bass as bass
import concourse.bacc as bacc
import concourse.mybir as mybir
from concourse import tile
from concourse.bass_utils import run_bass_kernel_spmd

B, S, DIM, FF, BLK = 64, 577, 768, 3072, 8
NCORES = 8
TOK = B * S                # 36928
T = TOK // NCORES          # 4616 tokens per core
P = 128
KD = DIM // P              # 6 contraction tiles for fc1 / output tiles fc2
KF = FF // P               # 24 ff tiles
CW = 512                   # chunk width (one PSUM bank of fp32)
F32 = mybir.dt.float32
BF16 = mybir.dt.bfloat16
GELU = mybir.ActivationFunctionType.Gelu

# 8x512 + 344 + 176 = 4616; all chunks wide enough to amortize dispatch,
# small final chunk shortens the post-compute drain tail
CHUNKS = [512] * 8 + [344, 176]
assert sum(CHUNKS) == T

NF, ND, NO = FF // BLK, DIM // BLK, DIM // BLK   # 384, 96, 96 blocks
FT, DT, OT = KF, KD, KD                          # 24, 6, 6 tiles
BPT = P // BLK                                   # 16 blocks per tile


# ---------------------------------------------------------------------------
# Host-side permutation search: pack mask-dead blocks into whole zero tiles.
# ---------------------------------------------------------------------------

def _greedy_fgroups(dead1, rng=None):
    """dead1 [NF, DT] bool -> f-block -> m-tile, packing whole dead tiles."""
    fg = -np.ones(NF, np.int32)
    tilei = 0
    remaining = np.ones(NF, bool)
    # tiles dead for a PAIR of k's first (worth 2 skips each)
    pairs = [(ka, kb) for ka in range(DT) for kb in range(ka + 1, DT)]
    if rng is not None:
        rng.shuffle(pairs)
    for ka, kb in pairs:
        while tilei < FT:
            cand = np.where(remaining & dead1[:, ka] & dead1[:, kb])[0]
            if len(cand) < BPT:
                break
            pick = cand[:BPT]
            fg[pick] = tilei
            remaining[pick] = False
            tilei += 1
    # single-k tiles, k by descending availability
    while tilei < FT:
        counts = sorted(((dead1[remaining, k].sum(), k) for k in range(DT)),
                        reverse=True)
        n, k = counts[0]
        if n < BPT:
            break
        cand = np.where(remaining & dead1[:, k])[0]
        other = dead1[cand].sum(1)   # prefer blocks with fewest other dead-k
        if rng is not None:
            order = np.argsort(other + rng.random(len(cand)) * 0.5)
        else:
            order = np.argsort(other, kind="stable")
        pick = cand[order][:BPT]
        fg[pick] = tilei
        remaining[pick] = False
        tilei += 1
    left = np.where(remaining)[0]
    pos = 0
    for t in range(FT):
        space = BPT - int((fg == t).sum())
        if space > 0:
            fg[left[pos:pos + space]] = t
            pos += space
    return fg


def _greedy_ogroups(dead2):
    """dead2 [NO, FT] bool -> o-block -> o-tile (6 tiles of 16)."""
    og = -np.ones(NO, np.int32)
    remaining = np.ones(NO, bool)
    tilei = 0
    counts = sorted(((dead2[:, ft].sum(), ft) for ft in range(FT)),
                    reverse=True)
    for n, ft in counts:
        if tilei >= OT:
            break
        cand = np.where(remaining & dead2[:, ft])[0]
        if len(cand) < BPT:
            continue
        other = dead2[cand].sum(1)
        pick = cand[np.argsort(other, kind="stable")][:BPT]
        og[pick] = tilei
        remaining[pick] = False
        tilei += 1
    left = np.where(remaining)[0]
    pos = 0
    for t in range(OT):
        space = BPT - int((og == t).sum())
        if space > 0:
            og[left[pos:pos + space]] = t
            pos += space
    return og


def _count_alive(m1, m2, fg, dg, og):
    nz1 = np.stack([m1[:, dg == k].sum(1) for k in range(DT)], 1)
    nz2 = np.stack([m2[:, fg == t].sum(1) for t in range(FT)], 1)
    alive1 = np.stack([(nz1[fg == t] > 0).sum(0) for t in range(FT)])
    alive2 = np.stack([(nz2[og == t] > 0).sum(0) for t in range(OT)])
    return alive1, alive2


def _repair(m1, m2, fg, dg, og, max_rounds=40):
    """Complete nearly-dead fc1 tiles via constrained f-block swaps that
    preserve every already-empty tile (fc1 and fc2)."""
    dead1 = np.stack([~m1[:, dg == k].any(1) for k in range(DT)], 1)
    for _ in range(max_rounds):
        alive1, alive2 = _count_alive(m1, m2, fg, dg, og)
        skipset1 = [set(np.where(alive1[t] == 0)[0]) for t in range(FT)]
        skipset2 = [set(np.where(alive2[t] == 0)[0]) for t in range(OT)]
        orows_req = [set() for _ in range(FT)]
        for ot in range(OT):
            for ft in skipset2[ot]:
                orows_req[ft].update(np.where(og == ot)[0])
        improved = False
        order = sorted(
            (int(alive1[mt, kt]), mt, kt)
            for mt in range(FT) for kt in range(DT)
            if 1 <= alive1[mt, kt] <= 3)
        for _na, mt, kt in order:
            alive_blocks = [f for f in np.where(fg == mt)[0]
                            if not dead1[f, kt]]
            req_mt = skipset1[mt] | {kt}
            oreq_mt = orows_req[mt]
            swaps, used_out, ok = [], set(), True
            for a in alive_blocks:
                found = None
                for b in np.where(dead1[:, kt])[0]:
                    if fg[b] == mt or b in used_out:
                        continue
                    tb = fg[b]
                    if not all(dead1[b, k2] for k2 in req_mt):
                        continue
                    if any(m2[o, b] for o in oreq_mt):
                        continue
                    if not all(dead1[a, k2] for k2 in skipset1[tb]):
                        continue
                    if any(m2[o, a] for o in orows_req[tb]):
                        continue
                    found = b
                    break
                if found is None:
                    ok = False
                    break
                used_out.add(found)
                swaps.append((a, found))
            if ok and swaps:
                for a, b in swaps:
                    fg[a], fg[b] = fg[b], fg[a]
                improved = True
                break
        if not improved:
            break
    return fg


def _pack(m1, m2, dg, restarts):
    """Given a d-grouping, pack f- and o-blocks; return (score, fg, og)."""
    dead1 = np.stack([~m1[:, dg == k].any(1) for k in range(DT)], 1)
    best = None
    for trial in range(restarts):
        rng = None if trial == 0 else np.random.default_rng(trial)
        fg = _greedy_fgroups(dead1, rng=rng)
        fg = _repair(m1, m2, fg, dg, np.arange(NO) // BPT)
        dead2 = np.stack([~m2[:, fg == t].any(1) for t in range(FT)], 1)
        og = _greedy_ogroups(dead2)
        fg = _repair(m1, m2, fg, dg, og)
        a1, a2 = _count_alive(m1, m2, fg, dg, og)
        score = int((a1 == 0).sum() + (a2 == 0).sum())
        if best is None or score > best[0]:
            best = (score, fg.copy(), og.copy())
    return best


_PLAN_CACHE = {}


def plan_permutation(mask1, mask2, restarts=30, dswap_seconds=90.0,
                     dswap_tries=9000):
    import time as _time
    m1 = np.asarray(mask1, bool)   # [NF, ND]
    m2 = np.asarray(mask2, bool)   # [NO, NF]
    ck = (m1.tobytes(), m2.tobytes())
    if ck in _PLAN_CACHE:
        return _PLAN_CACHE[ck]
    dg = np.arange(ND) // BPT      # natural d-grouping
    score, fg, og = _pack(m1, m2, dg, restarts)
    # hill-climb the d-grouping (deterministic rng; time+try boxed)
    rng = np.random.default_rng(123)
    deadline = _time.time() + dswap_seconds
    tries = 0
    while tries < dswap_tries and _time.time() < deadline:
        a, b = rng.integers(0, ND, 2)
        if dg[a] == dg[b]:
            continue
        tries += 1
        dg2 = dg.copy()
        dg2[a], dg2[b] = dg2[b], dg2[a]
        s2, fg2, og2 = _pack(m1, m2, dg2, restarts=3)
        if s2 >= score:
            dg = dg2
            if s2 > score:
                score, fg, og = s2, fg2, og2
    # final repack at full restarts with the best d-grouping
    s3, fg3, og3 = _pack(m1, m2, dg, restarts)
    if s3 > score:
        score, fg, og = s3, fg3, og3

    def perm(g, ntiles):
        return np.concatenate([np.where(g == t)[0] for t in range(ntiles)])

    fperm, dperm, operm = perm(fg, FT), perm(dg, DT), perm(og, OT)
    p1 = m1[np.ix_(fperm, dperm)]
    p2 = m2[np.ix_(operm, fperm)]
    skip1 = frozenset(
        (mt, kt) for mt in range(FT) for kt in range(DT)
        if not p1[mt*BPT:(mt+1)*BPT, kt*BPT:(kt+1)*BPT].any())
    skip2 = frozenset(
        (ot, ft) for ot in range(OT) for ft in range(FT)
        if not p2[ot*BPT:(ot+1)*BPT, ft*BPT:(ft+1)*BPT].any())

    def expand(p):
        return (p[:, None] * BLK + np.arange(BLK)[None, :]).ravel()

    return expand(fperm), expand(dperm), expand(operm), skip1, skip2


# ---------------------------------------------------------------------------
# Device program
# ---------------------------------------------------------------------------

def _body(tc, xt_d, w1_d, b1_d, w2_d, b2_d, o_d, skip1, skip2):
    nc = tc.nc
    with (
        tc.tile_pool(name="const", bufs=1) as constp,
        tc.tile_pool(name="wpool", bufs=1) as wp,
        tc.tile_pool(name="xt", bufs=2) as xtp,
        tc.tile_pool(name="h", bufs=2) as hp,
        tc.tile_pool(name="onat", bufs=2) as onatp,
        tc.tile_pool(name="ps1", bufs=3, space=bass.MemorySpace.PSUM) as ps1p,
        tc.tile_pool(name="ps2", bufs=3, space=bass.MemorySpace.PSUM) as ps2p,
    ):
        b1_s = constp.tile([P, KF], F32)
        nc.sync.dma_start(out=b1_s[:], in_=b1_d)
        b2_s = constp.tile([P, KD], F32)
        nc.sync.dma_start(out=b2_s[:], in_=b2_d)

        # w1 arrival order: the first two m-tiles' 128-col slices first (so
        # fc1 starts within ~2us), then the remainder in column-major
        # quarters. w2 streams concurrently on the scalar engine's DMA queue.
        w1_s = [wp.tile([P, FF], BF16, tag=f"w1_{k}", name=f"w1_{k}")
                for k in range(KD)]
        for m in range(2):
            for k in range(KD):
                nc.sync.dma_start(
                    out=w1_s[k][:, m * P:(m + 1) * P],
                    in_=w1_d[k * P:(k + 1) * P, m * P:(m + 1) * P],
                )
        W1Q = (FF - 2 * P) // 4
        for q in range(4):
            lo = 2 * P + q * W1Q
            hi = 2 * P + (q + 1) * W1Q
            for k in range(KD):
                nc.sync.dma_start(
                    out=w1_s[k][:, lo:hi],
                    in_=w1_d[k * P:(k + 1) * P, lo:hi],
                )
        w2_s = []
        for k in range(KF):
            w = wp.tile([P, DIM], BF16, tag=f"w2_{k}", name=f"w2_{k}")
            nc.scalar.dma_start(out=w[:], in_=w2_d[k * P:(k + 1) * P, :])
            w2_s.append(w)

        def load_x(c0, cw):
            xts = [xtp.tile([P, CW], BF16, tag=f"xt{k}", name=f"xt{k}")
                   for k in range(KD)]
            for k in range(KD):
                nc.gpsimd.dma_start(
                    out=xts[k][:, 0:cw],
                    in_=xt_d[k * P:(k + 1) * P, c0:c0 + cw],
                )
            return xts

        fc1_ks = [[k for k in range(KD) if (m, k) not in skip1] or [0]
                  for m in range(KF)]
        fc2_ks = [[k for k in range(KF) if (o, k) not in skip2] or [0]
                  for o in range(KD)]

        starts = [sum(CHUNKS[:i]) for i in range(len(CHUNKS))]
        xts = load_x(starts[0], CHUNKS[0])
        for ci, (c0, cw) in enumerate(zip(starts, CHUNKS)):
            # prefetch next chunk's xT while this chunk computes
            next_xts = (load_x(starts[ci + 1], CHUNKS[ci + 1])
                        if ci + 1 < len(CHUNKS) else None)

            # --- fc1: hT[m] = gelu(W1m slice.T @ xT + b1[m]) ---
            hts = []
            for m in range(KF):
                ps = ps1p.tile([P, CW], F32, tag="ps1", name="ps1")
                ks = fc1_ks[m]
                for k in ks:
                    nc.tensor.matmul(
                        ps[:, 0:cw],
                        w1_s[k][:, m * P:(m + 1) * P],
                        xts[k][:, 0:cw],
                        start=(k == ks[0]), stop=(k == ks[-1]),
                    )
                ht = hp.tile([P, CW], BF16, tag=f"h{m}", name=f"h{m}")
                nc.scalar.activation(
                    ht[:, 0:cw], ps[:, 0:cw], GELU, bias=b1_s[:, m:m + 1]
                )
                hts.append(ht)

            # --- fc2: outT[o] = W2m slice.T @ hT + b2[o] ---
            for o in range(KD):
                ps = ps2p.tile([P, CW], F32, tag="ps2", name="ps2")
                ks = fc2_ks[o]
                for k in ks:
                    nc.tensor.matmul(
                        ps[:, 0:cw],
                        w2_s[k][:, o * P:(o + 1) * P],
                        hts[k][:, 0:cw],
                        start=(k == ks[0]), stop=(k == ks[-1]),
                    )
                ot = onatp.tile([P, CW], BF16, tag=f"o{o}", name=f"o{o}")
                nc.vector.tensor_scalar_add(
                    ot[:, 0:cw], ps[:, 0:cw], b2_s[:, o:o + 1]
                )
                nc.sync.dma_start(
                    out=o_d[o * P:(o + 1) * P, c0:c0 + cw], in_=ot[:, 0:cw]
                )
            xts = next_xts


def build_program(skip1, skip2, t_tokens=T):
    nc = bacc.Bacc("TRN2", target_bir_lowering=False, debug=False,
                   num_devices=NCORES)
    xt_d = nc.dram_tensor("xt", [DIM, t_tokens], BF16,
                          kind="ExternalInput").ap()
    w1_d = nc.dram_tensor("w1t", [DIM, FF], BF16, kind="ExternalInput").ap()
    b1_d = nc.dram_tensor("b1", [P, KF], F32, kind="ExternalInput").ap()
    w2_d = nc.dram_tensor("w2t", [FF, DIM], BF16, kind="ExternalInput").ap()
    b2_d = nc.dram_tensor("b2", [P, KD], F32, kind="ExternalInput").ap()
    o_d = nc.dram_tensor("outt", [DIM, t_tokens], BF16,
                         kind="ExternalOutput").ap()
    with tile.TileContext(nc) as tc:
        _body(tc, xt_d, w1_d, b1_d, w2_d, b2_d, o_d, skip1, skip2)
    nc.compile()
    return nc


def host_prep(x, W1, b1, W2, b2, mask1, mask2, fpe, dpe, ope):
    m1 = np.repeat(np.repeat(np.asarray(mask1, dtype=bool), BLK, 0), BLK, 1)
    m2 = np.repeat(np.repeat(np.asarray(mask2, dtype=bool), BLK, 0), BLK, 1)
    xt = np.ascontiguousarray(
        np.asarray(x, np.float32).reshape(TOK, DIM).T[dpe]
    ).astype(ml_dtypes.bfloat16)                                  # [DIM, TOK]
    wm1t = (np.asarray(W1, np.float32) * m1.astype(np.float32)).T
    w1t = np.ascontiguousarray(
        wm1t[np.ix_(dpe, fpe)]).astype(ml_dtypes.bfloat16)        # [DIM, FF]
    wm2t = (np.asarray(W2, np.float32) * m2.astype(np.float32)).T
    w2t = np.ascontiguousarray(
        wm2t[np.ix_(fpe, ope)]).astype(ml_dtypes.bfloat16)        # [FF, DIM]
    b1h = np.ascontiguousarray(
        np.asarray(b1, np.float32)[fpe].reshape(KF, P).T)         # [P, KF]
    b2h = np.ascontiguousarray(
        np.asarray(b2, np.float32)[ope].reshape(KD, P).T)         # [P, KD]
    return xt, w1t, b1h, w2t, b2h


_PROGRAM = None
_PROGRAM_KEY = None


def _get_program(skip1, skip2, key):
    global _PROGRAM, _PROGRAM_KEY
    if _PROGRAM is None or _PROGRAM_KEY != key:
        _PROGRAM = build_program(skip1, skip2)
        _PROGRAM_KEY = key
    return _PROGRAM


def kernel(x, W1, b1, W2, b2, mask1, mask2, **run_kwargs):
    fpe, dpe, ope, skip1, skip2 = plan_permutation(mask1, mask2)
    xt, w1t, b1h, w2t, b2h = host_prep(
        x, W1, b1, W2, b2, mask1, mask2, fpe, dpe, ope)
    key = (np.asarray(mask1).tobytes(), np.asarray(mask2).tobytes())
    nc = _get_program(skip1, skip2, key)
    in_maps = [
        {"xt": np.ascontiguousarray(xt[:, c * T:(c + 1) * T]),
         "w1t": w1t, "b1": b1h, "w2t": w2t, "b2": b2h}
        for c in range(NCORES)
    ]
    res = run_bass_kernel_spmd(nc, in_maps, list(range(NCORES)), **run_kwargs)
    outt = np.concatenate(
        [np.asarray(res.results[c]["outt"]) for c in range(NCORES)], axis=1
    )                                                             # [DIM, TOK]
    out = np.empty((TOK, DIM), np.float32)
    out[:, ope] = outt.T.astype(np.float32)
    out = out.reshape(B, S, DIM)
    if run_kwargs:
        kernel.last_results = res
    return out
